# revision 2
# baseline (speedup 1.0000x reference)
"""Trainium2 Bass kernel for a 2-layer LSTM encoder (relu cell activation). v3

Problem: x[128, 512, 64] -> LSTM(256, relu, seq) -> LSTM(128, relu, last) -> out[128, 128]

v3 vs v2: the step is latency-bound (serial chain h -> MMs -> sigmoid ->
cell update -> h), so the tail is restructured to shorten the chain:
  - G gates go through PSUM via a combined inject (one id8 MM per bank),
    removing v2's g_full DVE op from the chain.
  - One sigmoid per bank (IF / O / L2) instead of gif+go serialization;
    banks are split so sigmoid(IF) fires right after the 8 IF matmuls
    instead of after the whole sweep.
  - f*c runs on GPSIMD in parallel with i*relu(g) on DVE; DVE chain is
    ig -> c -> h (in-order, no intermediate semaphores).
  - Layer 2 is a separate sub-chain (own PSUM bank, own sigmoid/update
    ops) riding in the slack of the L1 cycle instead of widening it.
"""

import numpy as np
from contextlib import ExitStack

import concourse.bass as bass
import concourse.tile as tile
from concourse import bacc
from concourse import mybir
from concourse.bass_utils import run_bass_kernel_spmd

fp32 = mybir.dt.float32
bf16 = mybir.dt.bfloat16
fp8 = mybir.dt.float8e4
AF = mybir.ActivationFunctionType
AOP = mybir.AluOpType

_STRIP_ENGINES = ("PE", "DVE", "Activation", "Pool")


def _strip_self_waits(nc):
    """Remove semaphore waits that target the waiting engine's own
    sole-updater counter when program order already guarantees the value.

    Each engine executes its instruction stream strictly in order and runs
    ops to completion (DVE/ACT drain between ops; PE matmuls are pc-monotone
    in start and end), so a wait on the engine's own counter for a value
    covered by preceding same-engine increments can never be unsatisfied at
    execution time. It only adds the increment-propagation latency (~100ns)
    to every producer->consumer hop within one engine. Cross-engine waits
    are untouched.
    """
    fn = nc.m.functions[0]
    blocks = list(fn.blocks)
    # classify sems by sole updating engine
    updaters = {}
    for b in blocks:
        for inst in b.instructions:
            si = inst.sync_info
            if si is None:
                continue
            eng = str(inst.engine).split(".")[-1]
            for u in (si.on_update or []):
                prev = updaters.setdefault(u.id, eng)
                if prev != eng:
                    updaters[u.id] = None  # multi-engine: off limits
    cum = {}
    n_stripped = 0
    for b in blocks:
        for inst in b.instructions:
            si = inst.sync_info
            if si is None:
                continue
            eng = str(inst.engine).split(".")[-1]
            new_waits = []
            for w in (si.on_wait or []):
                drop = (
                    eng in _STRIP_ENGINES
                    and w.wait_mode == "sem-ge-imm"
                    and updaters.get(w.id) == eng
                    and w.wait_value <= cum.get(w.id, 0)
                )
                if drop:
                    n_stripped += 1
                else:
                    new_waits.append(w)
            if len(new_waits) != len(si.on_wait or []):
                inst.sync_info = mybir.SyncInfo(
                    on_wait=new_waits, on_update=list(si.on_update or []))
            for u in (si.on_update or []):
                if u.update_mode in ("sem-inc", "sem-add-imm"):
                    cum[u.id] = cum.get(u.id, 0) + u.update_value
    return n_stripped

B, T, F = 128, 512, 64
U1, U2 = 256, 128
NCORES = 8
BL = B // NCORES  # 16 batch rows per core
S = 16.0  # fp8 weight scale

# gate block order in fused layout: I, F, O, G. Keras weight column order is
# i, f, g, o -> column offsets per block:
COLMAP1 = [0 * U1, 1 * U1, 3 * U1, 2 * U1]
COLMAP2 = [0 * U2, 1 * U2, 3 * U2, 2 * U2]
# zx column base per L1 block (I, F, O, G), each block 2*16 wide; L2 at 128.
Z1 = [0, 32, 64, 96]

CB8_COLS = 8 * U1 + 4 * U2 + 128   # u1q(2x1024) | u2q(512) | id8(128)
CB16_COLS = 8 * U2 + 1024          # w2q(2x512) | w1(1024, rows 0:64)
CF32_COLS = 128 + 8 + 4            # idf(128) | b1p(8) | b2p(4)


def build(T_=T, CH=32, nonzero_bias=False, reps=1):
    assert T_ % CH == 0 and CH % 2 == 0
    NCH = T_ // CH
    HCH = CH // 2
    RING = 3 * CH
    LAG = CH
    TOT = T_ + LAG

    nc = bacc.Bacc("TRN2", target_bir_lowering=False, debug=False)

    xt_d = nc.declare_dram_parameter("xt", [F, T_ * BL], bf16, isOutput=False)
    c8_d = nc.declare_dram_parameter("cb8", [128, CB8_COLS], fp8, isOutput=False)
    cb_d = nc.declare_dram_parameter("cb16", [128, CB16_COLS], bf16, isOutput=False)
    cf_d = nc.declare_dram_parameter("cf32", [128, CF32_COLS], fp32, isOutput=False)
    out_d = nc.declare_dram_parameter("out", [BL, U2], fp32, isOutput=True)

    with tile.TileContext(nc) as tc, ExitStack() as ctx:
        const_p = ctx.enter_context(tc.tile_pool(name="const", bufs=1))
        zx_p = ctx.enter_context(tc.tile_pool(name="zx", bufs=3))
        ew_p = ctx.enter_context(tc.tile_pool(name="ew", bufs=3))
        state_p = ctx.enter_context(tc.tile_pool(name="state", bufs=1))
        pif_p = ctx.enter_context(tc.tile_pool(name="pif", bufs=2, space="PSUM"))
        pl2_p = ctx.enter_context(tc.tile_pool(name="pl2", bufs=2, space="PSUM"))
        pb_p = ctx.enter_context(tc.tile_pool(name="pb", bufs=2, space="PSUM"))

        # ---- constants (one DMA each) ----
        c8 = const_p.tile([128, CB8_COLS], fp8, name="c8")
        nc.sync.dma_start(c8[:, :], c8_d[:, :])
        cb = const_p.tile([128, CB16_COLS], bf16, name="cb")
        nc.sync.dma_start(cb[:, :], cb_d[:, :])
        cf = const_p.tile([128, CF32_COLS], fp32, name="cf")
        nc.sync.dma_start(cf[:, :], cf_d[:, :])
        xt = const_p.tile([F, T_ * BL], bf16, name="xt")
        nc.sync.dma_start(xt[:, :], xt_d[:, :])

        u1q = [c8[:, 0:4 * U1], c8[:, 4 * U1:8 * U1]]
        u2q = c8[0:U2, 8 * U1:8 * U1 + 4 * U2]
        id8 = c8[:, 8 * U1 + 4 * U2:8 * U1 + 4 * U2 + 128]
        w2q = [cb[:, 0:4 * U2], cb[:, 4 * U2:8 * U2]]
        w1sb = cb[0:F, 8 * U2:8 * U2 + 1024]
        idf = cf[:, 0:128]
        b1sb = cf[:, 128:136]
        b2sb = cf[:, 136:140]

        # ---- persistent state ----
        c1 = state_p.tile([128, 2, 16], fp32)   # L1 cell (uc0 | uc1)
        c2 = state_p.tile([128, 16], fp32)      # L2 cell
        h_ring = state_p.tile([128, RING, 48], bf16)  # h/S: kc0|kc1|l2

        tc.strict_bb_all_engine_barrier()

        zx_tiles = [None] * (NCH + 1)

        def _get_zx(k):
            if zx_tiles[k] is None:
                z = zx_p.tile([128, CH, 192], bf16, name="zx", tag="zx")
                zx_tiles[k] = z
                if k == 0 or k >= NCH:
                    nc.vector.memset(z[:, :, :], 0.0)
            return zx_tiles[k]

        def _copy(j, dst, src, bias_ap):
            if nonzero_bias:
                nc.vector.tensor_scalar_add(dst, src, bias_ap)
            elif j % 2 == 0:
                nc.scalar.copy(dst, src)
            else:
                nc.vector.tensor_copy(dst, src)

        def l1x_piece(k, p):
            """x-part of L1 gates for chunk k, piece p = (bi, uc)."""
            bi, uc = p // 2, p % 2
            zk = _get_zx(k)
            cc = COLMAP1[bi] + uc * 128
            for sj in range(2):
                pb = pb_p.tile([128, HCH * BL], fp32, name="pb", tag="pb")
                nc.tensor.matmul(
                    pb[:, :], w1sb[:, cc:cc + 128],
                    xt[:, (k * CH + sj * HCH) * BL:(k * CH + (sj + 1) * HCH) * BL],
                    start=True, stop=True)
                _copy(p + sj, zk[:, sj * HCH:(sj + 1) * HCH,
                                 Z1[bi] + uc * 16:Z1[bi] + (uc + 1) * 16],
                      pb.rearrange("p (t b) -> p t b", b=BL),
                      b1sb[:, bi * 2 + uc:bi * 2 + uc + 1])

        def l2x_half(j, sj):
            """W2.T @ h1[chunk j, half sj] -> zx[j+1] L2 cols, steps half sj."""
            zk = _get_zx(j + 1)
            rs = (j * CH + sj * HCH) % RING
            for bi in range(4):
                pb = pb_p.tile([128, HCH * BL], fp32, name="pb2", tag="pb")
                for kc in range(2):
                    nc.tensor.matmul(
                        pb[:, :],
                        w2q[kc][:, COLMAP2[bi]:COLMAP2[bi] + 128],
                        h_ring[:, rs:rs + HCH, kc * 16:(kc + 1) * 16],
                        start=(kc == 0), stop=(kc == 1))
                _copy(bi, zk[:, sj * HCH:(sj + 1) * HCH,
                             128 + bi * 16:128 + (bi + 1) * 16],
                      pb.rearrange("p (t b) -> p t b", b=BL),
                      b2sb[:, bi:bi + 1])

        def emit_body():
            nonlocal h2f
            zx_tiles[:] = [None] * (NCH + 1)
            for p in range(8):
                l1x_piece(0, p)
            for t in range(TOT):
                k, tl = divmod(t, CH)
                s = t - LAG  # layer-2 step
                if tl == 1 and 1 <= k <= NCH:
                    l2x_half(k - 1, 1)
                if tl == HCH + 4 and k < NCH:
                    l2x_half(k, 0)
                if tl in (2, 4, 6, 8, 10, 12, 14, 16) and k + 1 < NCH:
                    l1x_piece(k + 1, (tl - 2) // 2)

                zxt = zx_tiles[k]
                hp = h_ring[:, (t - 1) % RING, :]

                pz = None
                if t < T_:
                    pz = pif_p.tile([128, 8, 16], fp32, name="pz")
                pz_l2 = None
                if s >= 0:
                    pz_l2 = pl2_p.tile([128, 4, 16], fp32, name="pz_l2")

                # injects first: no h dependency, prefire during prev tail
                if t < T_:
                    nc.tensor.matmul(pz[:, :, :], id8[:, :],
                                     zxt[:, tl, 0:128], start=True, stop=False)
                if s >= 0:
                    nc.tensor.matmul(pz_l2[:, :, :], id8[:, :],
                                     zxt[:, tl, 128:192], start=True, stop=False)

                if t < T_:
                    # L1 bank: 16 MMs (kc x {I,F,O,G} x uc)
                    n = 0
                    for kc in range(2):
                        for bi in range(4):
                            for uc in range(2):
                                cc = COLMAP1[bi] + uc * 128
                                n += 1
                                nc.tensor.matmul(
                                    pz[:, bi * 2 + uc, :],
                                    u1q[kc][:, cc:cc + 128],
                                    hp[:, kc * 16:(kc + 1) * 16],
                                    start=False, stop=(n == 16))
                if s >= 0:
                    for n, bi in enumerate((0, 1, 2, 3)):
                        nc.tensor.matmul(
                            pz_l2[:, bi, :],
                            u2q[:, COLMAP2[bi]:COLMAP2[bi] + 128],
                            hp[:, 32:48], start=False, stop=(n == 3))

                # ---- tail ----
                # L1 chain ops get scheduler priority so step-local L2/bulk
                # work cannot slot in front of them on ACT/DVE.
                if t < T_:
                    gifo = ew_p.tile([128, 6, 16], fp32, name="gifo")
                    with tc.high_priority(offset=80):
                        nc.scalar.activation(
                            gifo[:, :, :], pz[:, 0:6, :], AF.Sigmoid)
                gl2 = None
                if s >= 0:
                    gl2 = ew_p.tile([128, 3, 16], fp32, name="gl2")
                    nc.scalar.activation(gl2[:, :, :], pz_l2[:, 0:3, :], AF.Sigmoid)

                if t < T_:
                    # f*c, i*relu(g), c, h all on DVE back-to-back: one
                    # ACT->DVE handoff, no intermediate semaphores.
                    fc = ew_p.tile([128, 2, 16], fp32, name="fc")
                    ig = ew_p.tile([128, 2, 16], fp32, name="ig")
                    with tc.high_priority(offset=80):
                        nc.vector.tensor_mul(fc[:, :, :], gifo[:, 2:4, :], c1[:, :, :])
                        nc.vector.scalar_tensor_tensor(
                            ig[:, :, :], pz[:, 6:8, :], 0.0, gifo[:, 0:2, :],
                            AOP.max, AOP.mult)
                        nc.vector.tensor_add(c1[:, :, :], ig[:, :, :], fc[:, :, :])
                        slot = t % RING
                        h1_inst = nc.vector.scalar_tensor_tensor(
                            h_ring[:, slot, 0:32], gifo[:, 4:6, :], 1.0 / S,
                            c1[:, :, :], AOP.mult, AOP.mult)
                if s >= 0:
                    ig2 = ew_p.tile([128, 16], fp32, name="ig2")
                    ig2_inst = nc.vector.scalar_tensor_tensor(
                        ig2[:, :], pz_l2[:, 3, :], 0.0, gl2[:, 0, :],
                        AOP.max, AOP.mult)
                    if t < T_:
                        # scheduler-only edge: keep the L1 critical-path h
                        # write ahead of L2 tail work in the DVE stream
                        ig2_inst.ins.add_dependency(
                            h1_inst.ins.name,
                            mybir.DependencyInfo(sync=False, no_sync=True))
                    fc2 = ew_p.tile([128, 16], fp32, name="fc2")
                    nc.gpsimd.tensor_mul(fc2[:, :], gl2[:, 1, :], c2[:, :])
                    nc.vector.tensor_add(c2[:, :], ig2[:, :], fc2[:, :])
                    slot = t % RING
                    nc.vector.scalar_tensor_tensor(
                        h_ring[:, slot, 32:48], gl2[:, 2, :], 1.0 / S,
                        c2[:, :], AOP.mult, AOP.mult)

                if t == LAG - 1:
                    slot = t % RING
                    nc.vector.memset(h_ring[:, slot, 32:48], 0.0)
                    nc.vector.memset(c2[:, :], 0.0)
                if t == TOT - 1:
                    h2f = ew_p.tile([128, BL], fp32, name="h2f")
                    nc.vector.tensor_mul(h2f[:, :], gl2[:, 2, :], c2[:, :])

        h2f = None
        for _rep in range(reps):
            nc.vector.memset(c1[:, :, :], 0.0)
            nc.vector.memset(c2[:, :], 0.0)
            nc.vector.memset(h_ring[:, RING - 1, :], 0.0)
            emit_body()

        pfin = pb_p.tile([BL, 128], fp32, name="pfin", tag="pb")
        nc.tensor.transpose(pfin[:, :], h2f[:, :], idf[:, :])
        osb = ew_p.tile([BL, 128], fp32, name="osb")
        nc.scalar.copy(osb[:, :], pfin[:, :])
        nc.sync.dma_start(out_d[:, :], osb[:, :])

    import os
    if os.environ.get("STRIP") == "1":
        _strip_self_waits(nc)
    nc.finalize()
    return nc


_cache = {}


def _get_nc(T_=T, CH=32, nonzero_bias=False, reps=1):
    key = (T_, CH, nonzero_bias, reps)
    if key not in _cache:
        _cache[key] = build(T_, CH, nonzero_bias, reps)
    return _cache[key]


def make_inputs(x, W1, U1w, b1, W2, U2w, b2, T_=T):
    np8 = mybir.dt.np(fp8)
    npb = mybir.dt.np(bf16)
    x = np.asarray(x, np.float32)
    W1 = np.asarray(W1, np.float32)
    U1w = np.asarray(U1w, np.float32)
    W2 = np.asarray(W2, np.float32)
    U2w = np.asarray(U2w, np.float32)
    b1 = np.asarray(b1, np.float32)
    b2 = np.asarray(b2, np.float32)

    cb8 = np.zeros((128, CB8_COLS), np8)
    u1q = (U1w * S).astype(np8)
    cb8[:, 0:1024] = u1q[0:128]
    cb8[:, 1024:2048] = u1q[128:256]
    cb8[:, 2048:2560] = (U2w * S).astype(np8)
    cb8[:, 2560:2688] = np.eye(128, dtype=np.float32).astype(np8)

    cb16 = np.zeros((128, CB16_COLS), npb)
    w2q = (W2 * S).astype(npb)
    cb16[:, 0:512] = w2q[0:128]
    cb16[:, 512:1024] = w2q[128:256]
    cb16[0:64, 1024:2048] = W1.astype(npb)

    b1p = np.zeros((128, 8), np.float32)
    for bi in range(4):
        for uc in range(2):
            b1p[:, bi * 2 + uc] = b1[COLMAP1[bi] + uc * 128:COLMAP1[bi] + (uc + 1) * 128]
    b2p = np.zeros((128, 4), np.float32)
    for bi in range(4):
        b2p[:, bi] = b2[COLMAP2[bi]:COLMAP2[bi] + 128]
    cf32 = np.zeros((128, CF32_COLS), np.float32)
    cf32[:, 0:128] = np.eye(128, dtype=np.float32)
    cf32[:, 128:136] = b1p
    cf32[:, 136:140] = b2p

    common = dict(cb8=cb8, cb16=cb16, cf32=cf32)
    xr = x.reshape(NCORES, BL, x.shape[1], F)
    in_maps = []
    for c in range(NCORES):
        xtc = np.ascontiguousarray(
            xr[c][:, :T_].transpose(2, 1, 0).reshape(F, T_ * BL)).astype(npb)
        m = dict(common)
        m["xt"] = xtc
        in_maps.append(m)
    nonzero_bias = bool(np.any(b1) or np.any(b2))
    return in_maps, nonzero_bias


def run(inputs, T_=T, CH=32, trace=False, reps=1):
    in_maps, nzb = make_inputs(
        inputs["x"], inputs["W1"], inputs["U1"], inputs["b1"],
        inputs["W2"], inputs["U2"], inputs["b2"], T_=T_)
    nc = _get_nc(T_, CH, nzb, reps)
    res = run_bass_kernel_spmd(nc, in_maps, list(range(NCORES)), trace=trace)
    out = np.concatenate(
        [res.results[c]["out"] for c in range(NCORES)], axis=0)
    return np.ascontiguousarray(out, dtype=np.float32), res.exec_time_ns


def kernel(x, W1, U1, b1, W2, U2, b2):
    out, _ = run(dict(x=x, W1=W1, U1=U1, b1=b1, W2=W2, U2=U2, b2=b2))
    return out
